# revision 1
# baseline (speedup 1.0000x reference)
"""Trainium2 Bass kernel for nn_MultiHeadAttention_72765335929011.

Reference computation (per batch b, query q):
    q    = h @ Wq.T + bq                     (B, Lq, H*Dk)
    k    = h_arc @ Wk.T + bk                 (B, Lq, Lk, H*Dk)
    v    = h_arc @ Wv.T + bv
    s    = einsum('hd,khd->hk', q_, k_) / sqrt(D)
    attn = softmax(mask ? s : -1e9)
    ctx  = einsum('hk,khd->hd', attn, v)
    out  = ctx @ Wo.T + bo

Key restructure (exact algebra): with Dk=32 per head, fold Wk into the
query side and Wv@Wo into a per-head matrix F:
    qk[h,:]   = Wk_h.T @ q_h          -> scores = qk . h_arc  (i-contraction)
    ctxm[h,:] = attn @ h_arc          (k-contraction, model space)
    out       = sum_h ctxm_h @ F_h.T + (Wo@bv + bo),  F_h = Wo_h @ Wv_h
bk cancels in softmax (constant row shift); bv reduces to Wo@bv since
attn rows sum to 1. This cuts FLOPs ~15x and reads h_arc exactly once.

Dataflow per macro-tile (16 queries, one (b,q)-slab of h_arc):
  slab f32 --cast--> slab_h fp16 --PE transpose--> Tp (i-part) for scores;
  scoresT (k-part, q*8+h free) accumulates mask (lmask.T @ Bsel) plus
  per-q qk matmuls; exp on ACT gives eT directly; denominators via a
  ones-matmul partition reduction; ctxm = eT.T @ slab_q with row
  extraction fused with the 1/denom scale. fp16 operands, fp32 PSUM.

Sharding: data-parallel over B across 8 cores (2 batches each), no
collectives.
"""

import sys

import numpy as np

sys.path.insert(0, "/opt/trn_rl_repo")

import concourse.bass as bass
import concourse.mybir as mybir
import concourse.tile as tile
from concourse import bacc
from concourse.bass_utils import run_bass_kernel_spmd
from concourse.masks import make_identity

F32 = mybir.dt.float32
F16 = mybir.dt.float16
I32 = mybir.dt.int32

B, LQ, LK, D = 16, 128, 128, 256
H, DKV = 8, 32
HD = H * DKV  # 256
NCORES = 8
BLOC = B // NCORES  # batches per core
N = BLOC * LQ  # 256 query rows per core
GQ = 16  # queries per macro-tile (GQ*H = 128)
NMACRO = N // GQ  # 16
# exp(scale*(s + MASK_RAW)) with scale=1/16 -> exp(s/16 - 50) ~ 2e-22 for
# masked slots (flushes to 0 in fp16); an all-masked row normalizes to
# uniform exactly like the reference.
MASK_RAW = -800.0
SCALE = 1.0 / 16.0  # 1/sqrt(D)


def _build_program(loop_reps=1):
    nc = bacc.Bacc(
        "TRN2",
        target_bir_lowering=False,
        debug=False,
        enable_asserts=True,
        num_devices=NCORES,
    )

    h_d = nc.dram_tensor("h", (BLOC, LQ, D), F32, kind="ExternalInput").ap()
    harc_d = nc.dram_tensor("h_arc", (BLOC, LQ, LK, D), F32, kind="ExternalInput").ap()
    mask_d = nc.dram_tensor("mask", (BLOC, LQ, LK), I32, kind="ExternalInput").ap()
    wq_d = nc.dram_tensor("Wq", (HD, D), F32, kind="ExternalInput").ap()
    wk_d = nc.dram_tensor("Wk", (HD, D), F32, kind="ExternalInput").ap()
    wv_d = nc.dram_tensor("Wv", (HD, D), F32, kind="ExternalInput").ap()
    wo_d = nc.dram_tensor("Wo", (D, HD), F32, kind="ExternalInput").ap()
    bq_d = nc.dram_tensor("bq", (HD,), F32, kind="ExternalInput").ap()
    bk_d = nc.dram_tensor("bk", (HD,), F32, kind="ExternalInput").ap()  # unused
    bv_d = nc.dram_tensor("bv", (HD,), F32, kind="ExternalInput").ap()
    bo_d = nc.dram_tensor("bo", (D,), F32, kind="ExternalInput").ap()
    # host-supplied constants (walrus codegen chokes on some affine_selects)
    id32_d = nc.dram_tensor("c_id32", (128, 128), F32, kind="ExternalInput").ap()
    id16_d = nc.dram_tensor("c_id16", (128, 128), F16, kind="ExternalInput").ap()
    bsel_d = nc.dram_tensor("c_bsel", (16, 128), F16, kind="ExternalInput").ap()
    ones_d = nc.dram_tensor("c_ones", (128, 128), F16, kind="ExternalInput").ap()
    out_d = nc.dram_tensor("out", (BLOC, LQ, D), F32, kind="ExternalOutput").ap()

    with tile.TileContext(nc) as tc:
        _emit(tc, h_d, harc_d, mask_d, wq_d, wk_d, wv_d, wo_d, bq_d, bv_d, bo_d,
              id32_d, id16_d, bsel_d, ones_d, out_d, loop_reps=loop_reps)

    nc.compile()
    return nc


def _emit(tc, h_d, harc_d, mask_d, wq_d, wk_d, wv_d, wo_d, bq_d, bv_d, bo_d,
          id32_d, id16_d, bsel_d, ones_d, out_d, loop_reps=1):
    import contextlib
    import os as _os
    _STAGE = int(_os.environ.get("KSTAGE", "9"))
    nc = tc.nc
    acopy = nc.any.tensor_copy

    with (
        tc.tile_pool(name="const", bufs=1) as constp,
        tc.tile_pool(name="wts", bufs=1) as wts,
        tc.tile_pool(name="big", bufs=1) as big,
        tc.tile_pool(name="slab", bufs=3) as slabp,
        tc.tile_pool(name="tpb", bufs=10) as tpbp,
        tc.tile_pool(name="sm", bufs=2) as smp,
        tc.tile_pool(name="tp_ps", bufs=3, space=bass.MemorySpace.PSUM) as tp_ps,
        tc.tile_pool(name="sc_ps", bufs=2, space=bass.MemorySpace.PSUM) as sc_ps,
        tc.tile_pool(name="mm_ps", bufs=1, space=bass.MemorySpace.PSUM) as mm_ps,
        tc.tile_pool(name="cx_ps", bufs=1, space=bass.MemorySpace.PSUM) as cx_ps,
    ):
        # ---------------- constants (DMA'd from host) ----------------
        ident = constp.tile([128, 128], F32, tag="ident", name="ident")
        nc.sync.dma_start(out=ident[:, :], in_=id32_d[:, :])
        ident_h = constp.tile([128, 128], F16, tag="ident_h", name="ident_h")
        nc.sync.dma_start(out=ident_h[:, :], in_=id16_d[:, :])

        # Bsel[q, y] = 1 iff y//8 == q  (16 x 128): lmask.T @ Bsel broadcasts
        # the per-(q,k) mask term to all 8 head columns of q.
        bsel = constp.tile([16, 128], F16, tag="bsel", name="bsel")
        nc.sync.dma_start(out=bsel[:, :], in_=bsel_d[:, :])

        onesf = constp.tile([128, 128], F16, tag="onesf", name="onesf")
        nc.sync.dma_start(out=onesf[:, :], in_=ones_d[:, :])
        ones1 = onesf[0:1, :]
        ones_col = onesf[:, 0:1]
        mbias = constp.tile([16, 1], F32, tag="mbias", name="mbias")
        nc.gpsimd.memset(mbias[:, :], MASK_RAW)
        zbias = constp.tile([128, 1], F32, tag="zbias", name="zbias")
        nc.gpsimd.memset(zbias[:, :], 0.0)

        # ---------------- weight / input staging (fp32 -> fp16) ----------------
        def load2h(name, dram):  # (256, 256) f32 -> two (128, 256) fp16 tiles
            out = []
            for c in range(2):
                t32 = wts.tile([128, D], F32, tag="stg32", name=f"{name}{c}_32")
                nc.sync.dma_start(out=t32[:, :], in_=dram[c * 128:(c + 1) * 128, :])
                t16 = wts.tile([128, D], F16, tag=f"{name}{c}", name=f"{name}{c}")
                acopy(t16[:, :], t32[:, :])
                out.append(t16)
            return out

        wq_s = load2h("wq", wq_d)
        wo_s = load2h("wo", wo_d)

        # Wk/Wv per-head at base partition 0: (32 d, h*256 + i), fp16.
        # (Matmul operands must sit at partition base 0/32/64.)
        def loadph(name, dram):
            t32 = wts.tile([32, H * D], F32, tag="stgph", name=f"{name}_32")
            nc.sync.dma_start(
                out=t32.rearrange("d (h i) -> d h i", h=H),
                in_=dram.rearrange("(h d) i -> d h i", d=DKV),
            )
            t16 = wts.tile([32, H * D], F16, tag=name, name=name)
            acopy(t16[:, :], t32[:, :])
            return t16

        wk2 = loadph("wk2", wk_d)
        wv2 = loadph("wv2", wv_d)

        h_s = []
        for b in range(BLOC):
            t32 = wts.tile([128, D], F32, tag="stg32", name=f"h{b}_32")
            nc.sync.dma_start(out=t32[:, :], in_=h_d[b])
            t16 = wts.tile([128, D], F16, tag=f"h{b}", name=f"h{b}")
            acopy(t16[:, :], t32[:, :])
            h_s.append(t16)

        bq_s = constp.tile([32, H], F32, tag="bq", name="bq")
        nc.sync.dma_start(out=bq_s[:, :], in_=bq_d.rearrange("(h d) -> d h", d=DKV))
        bv32 = constp.tile([32, H], F32, tag="bv32", name="bv32")
        nc.sync.dma_start(out=bv32[:, :], in_=bv_d.rearrange("(h d) -> d h", d=DKV))
        bv_s = constp.tile([32, H], F16, tag="bv", name="bv")
        acopy(bv_s[:, :], bv32[:, :])
        bo_s = constp.tile([1, D], F32, tag="bo", name="bo")
        nc.sync.dma_start(out=bo_s[:, :], in_=bo_d.rearrange("(o d) -> o d", o=1))

        # fp16 transpose helper: (128,128) fp16 SBUF chunk -> fp16 SBUF dest
        def transpose_to(dst_ap, src_ap):
            tp = tp_ps.tile([128, 128], F16, tag="tp_ps", name="tp_ps")
            nc.tensor.transpose(tp[:, :], src_ap, ident_h[:, :])
            acopy(dst_ap, tp[:, :])

        # hT: (i 2x128, n=256);  WqT: (i 2x128, hd=256)   [fp16]
        h_t = [big.tile([128, N], F16, tag=f"ht{c}", name=f"ht{c}") for c in range(2)]
        for b in range(BLOC):
            for ic in range(2):
                transpose_to(
                    h_t[ic][:, b * 128:(b + 1) * 128],
                    h_s[b][:, ic * 128:(ic + 1) * 128],
                )
        wq_t = [big.tile([128, HD], F16, tag=f"wqt{c}", name=f"wqt{c}") for c in range(2)]
        for hc in range(2):
            for ic in range(2):
                transpose_to(
                    wq_t[ic][:, hc * 128:(hc + 1) * 128],
                    wq_s[hc][:, ic * 128:(ic + 1) * 128],
                )
        # WoT per-head at base 0: (32 d, h*256 + o), via 32-col strips.
        wo_t2 = big.tile([32, H * D], F16, tag="wot2", name="wot2")
        for oc in range(2):
            for hh in range(H):
                tp = tp_ps.tile([128, 128], F16, tag="tp_ps", name="tp_ps")
                nc.tensor.transpose(
                    tp[:32, :], wo_s[oc][:, hh * 32:(hh + 1) * 32], ident_h[:, :]
                )
                acopy(wo_t2[:, hh * D + oc * 128: hh * D + (oc + 1) * 128], tp[:32, :])

        # qT per head at base 0: (32 d, h*256 + n), fp16, +bq
        q_t2 = big.tile([32, H * N], F16, tag="qt2", name="qt2")
        for hh in range(H):
            qp = mm_ps.tile([32, N], F32, tag="mm_ps", name="mm_ps")
            for ic in range(2):
                nc.tensor.matmul(
                    qp[:, :],
                    wq_t[ic][:, hh * 32:(hh + 1) * 32],
                    h_t[ic][:, :],
                    start=(ic == 0), stop=(ic == 1),
                )
            nc.scalar.activation(
                q_t2[:, hh * N:(hh + 1) * N], qp[:, :],
                mybir.ActivationFunctionType.Identity,
                bias=bq_s[:, hh:hh + 1], scale=1.0,
            )

        # qkT[i, n*8+h] = sum_d Wk[h*32+d, i] * qT[h*32+d, n]   (fp16)
        qk_t = [big.tile([128, N * H], F16, tag=f"qkt{c}", name=f"qkt{c}")
                for c in range(2)]
        for hh in range(H):
            for ic in range(2):
                qkp = mm_ps.tile([128, N], F32, tag="mm_ps", name="mm_ps")
                nc.tensor.matmul(
                    qkp[:, :],
                    wk2[:, hh * D + ic * 128: hh * D + (ic + 1) * 128],
                    q_t2[:, hh * N:(hh + 1) * N],
                )
                acopy(
                    qk_t[ic].rearrange("p (n h) -> p n h", h=H)[:, :, hh],
                    qkp[:, :],
                )

        # F_hT[i, o] = sum_d Wv[h*32+d, i] * Wo[o, h*32+d]  (fp16)
        f_t = [big.tile([128, H * D], F16, tag=f"ft{c}", name=f"ft{c}")
               for c in range(2)]
        for hh in range(H):
            for ic in range(2):
                fp = mm_ps.tile([128, D], F32, tag="mm_ps", name="mm_ps")
                nc.tensor.matmul(
                    fp[:, :],
                    wv2[:, hh * D + ic * 128: hh * D + (ic + 1) * 128],
                    wo_t2[:, hh * D:(hh + 1) * D],
                )
                acopy(f_t[ic][:, hh * D:(hh + 1) * D], fp[:, :])

        # bo_eff = Wo @ bv + bo  (1, 256) fp16
        bop = mm_ps.tile([1, D], F32, tag="mm_ps", name="mm_ps")
        for hh in range(H):
            nc.tensor.matmul(
                bop[:, :], bv_s[:, hh:hh + 1], wo_t2[:, hh * D:(hh + 1) * D],
                start=(hh == 0), stop=(hh == H - 1),
            )
        boe32 = constp.tile([1, D], F32, tag="boe32", name="boe32")
        nc.vector.tensor_add(boe32[:, :], bop[:, :], bo_s[:, :])
        boe = constp.tile([1, D], F16, tag="boe", name="boe")
        acopy(boe[:, :], boe32[:, :])

        # ctxm_T accumulator: (i 2x128, n*8+h), fp16
        cx_t = [big.tile([128, N * H], F16, tag=f"cxt{c}", name=f"cxt{c}")
                for c in range(2)]

        # ---------------- main loop over macro-tiles ----------------
        if _STAGE < 2:
            return
        loop_cm = tc.For_i(0, loop_reps, 1) if loop_reps > 1 else contextlib.nullcontext()
        with loop_cm:
            _emit_macros(
                tc, harc_d, mask_d, _STAGE, acopy, slabp, smp, tpbp, tp_ps, sc_ps,
                mm_ps, cx_ps, ident, ident_h, bsel, onesf, mbias, zbias,
                qk_t, cx_t,
            )
        if _STAGE < 6:
            return
        # ---------------- output projection ----------------
        # out[n, o] = sum_h sum_i ctxm_T[i, n*8+h] * F_hT[i, o] + bo_eff
        for nck in range(BLOC):
            op = mm_ps.tile([128, D], F32, tag="mm_ps", name="mm_ps")
            first = True
            for hh in range(H):
                for ic in range(2):
                    lhs = cx_t[ic].rearrange("p (n h) -> p n h", h=H)[
                        :, nck * 128:(nck + 1) * 128, hh
                    ]
                    nc.tensor.matmul(
                        op[:, :], lhs, f_t[ic][:, hh * D:(hh + 1) * D],
                        start=first, stop=False,
                    )
                    first = False
            nc.tensor.matmul(
                op[:, :], ones1[:, :], boe[:, :], start=False, stop=True,
            )
            os_ = smp.tile([128, D], F32, tag="os", name="os")
            nc.scalar.copy(os_[:, :], op[:, :])
            nc.sync.dma_start(out=out_d[nck], in_=os_[:, :])


def _emit_macros(tc, harc_d, mask_d, _STAGE, acopy, slabp, smp, tpbp, tp_ps, sc_ps,
                 mm_ps, cx_ps, ident, ident_h, bsel, onesf, mbias, zbias,
                 qk_t, cx_t):
    nc = tc.nc
    ones_col = onesf[:, 0:1]
    if True:
        for m in range(NMACRO):
            b, q0 = m // (LQ // GQ), (m % (LQ // GQ)) * GQ

            slab = slabp.tile([128, GQ * D], F32, tag="slab", name="slab")
            # 4 dma_starts (one per q-group) spread across three descriptor
            # generators (SP/ACT HWDGE + Pool SWDGE) so the 1KB-run pattern's
            # descriptor work runs in parallel
            for gq, deng in enumerate((nc.sync, nc.scalar, nc.gpsimd, nc.sync)):
                deng.dma_start(
                    out=slab.rearrange("k (q i) -> k q i", q=GQ)[:, gq * 4:(gq + 1) * 4, :],
                    in_=harc_d[b, q0 + gq * 4:q0 + (gq + 1) * 4].rearrange("q k i -> k q i"),
                )
            slab_h = slabp.tile([128, GQ * D], F16, tag="slab_h", name="slab_h")
            # cast in q-group quarters (subtile deps let transposes of group g
            # start as soon as its quarter is cast); alternate DVE/ACT
            QW = GQ * D // 4
            for gq in range(4):
                eng = nc.vector.tensor_copy if gq % 2 == 0 else nc.scalar.copy
                eng(slab_h[:, gq * QW:(gq + 1) * QW], slab[:, gq * QW:(gq + 1) * QW])

            mi = smp.tile([16, LK], I32, tag="mi", name="mi")
            nc.gpsimd.dma_start(out=mi[:, :], in_=mask_d[b, q0:q0 + GQ, :])
            mf = smp.tile([16, LK], F32, tag="mf", name="mf")
            nc.vector.tensor_copy(mf[:, :], mi[:, :])
            lmask = smp.tile([16, LK], F16, tag="lmask", name="lmask")
            # (mask - 1) * |MASK_RAW|  ->  {MASK_RAW, 0}
            nc.scalar.activation(
                lmask[:, :], mf[:, :],
                mybir.ActivationFunctionType.Identity,
                bias=mbias[:, :], scale=-MASK_RAW,
            )

            # scoresT (k-part, 16q*8h free); mask first:
            # (lmask.T @ Bsel)[k, q*8+h] = lmask[q, k]
            sct = sc_ps.tile([128, 128], F32, tag="sc_ps", name="sc_ps")
            nc.tensor.matmul(
                sct[:, :], lmask[:, :], bsel[:, :], start=True, stop=False,
            )

            for ql in range(GQ):
                # both i-chunk transposes land in one PSUM tile -> one copy
                tpp = tp_ps.tile([128, 256], F16, tag="tp_ps", name="tp_ps")
                for ic in range(2):
                    nc.tensor.transpose(
                        tpp[:, ic * 128:(ic + 1) * 128],
                        slab_h[:, ql * D + ic * 128: ql * D + (ic + 1) * 128],
                        ident_h[:, :],
                    )
                tb = tpbp.tile([128, 256], F16, tag="tpb", name="tpb")
                # ~11/5 DVE/ACT split of PSUM->SBUF copies (ACT is ~2x slower)
                ceng = nc.scalar.copy if ql % 3 == 2 else nc.vector.tensor_copy
                ceng(tb[:, :], tpp[:, :])
                nq = m * GQ + ql
                for ic in range(2):
                    nc.tensor.matmul(
                        sct[:, ql * 8:(ql + 1) * 8],
                        tb[:, ic * 128:(ic + 1) * 128],
                        qk_t[ic][:, nq * 8:(nq + 1) * 8],
                        start=False, stop=(ql == GQ - 1 and ic == 1),
                    )

            if _STAGE < 3:
                continue
            # eT = exp(scoresT/16) stays in (k, qh) layout -> fp16
            et = smp.tile([128, 128], F16, tag="et", name="et")
            nc.scalar.activation(
                et[:, :], sct[:, :],
                mybir.ActivationFunctionType.Exp,
                bias=zbias[:, :], scale=SCALE,
            )
            # denom[qh] = sum_k eT[k, qh]; recip transposed to (128,1)
            dn = mm_ps.tile([1, 128], F32, tag="mm_ps", name="dn")
            nc.tensor.matmul(dn[:, :], ones_col[:, :], et[:, :])
            recip = smp.tile([1, 128], F32, tag="recip", name="recip")
            nc.vector.reciprocal(recip[:, :], dn[:, :])
            rpt = mm_ps.tile([128, 1], F32, tag="mm_ps", name="rpt")
            nc.tensor.transpose(rpt[:, :], recip[:, :], ident[0:1, 0:1])
            rpt_s = smp.tile([128, 1], F32, tag="rpt_s", name="rpt_s")
            nc.vector.tensor_copy(rpt_s[:, :], rpt[:, :])

            if _STAGE < 4:
                continue
            # ctxm: the pair (q=4g+2p, q=4g+2p+1) shares the eT g-block
            # stationary: out (32, 512) at block-row g*32, pair-column p of
            # cxp_all. Useful rows r = g*32 + j*8 + h (j = 2p+side) carry
            # q = 4g+j, whose packed recip index is exactly r, so a full-tile
            # rpt_s scale normalizes during extraction.
            cxp_all = cx_ps.tile([128, 4 * D], F32, tag="cx_ps", name="cx_ps")
            for g in range(4):
                for p in range(2):
                    ql = g * 4 + 2 * p
                    nc.tensor.matmul(
                        cxp_all[g * 32:(g + 1) * 32, p * 2 * D:(p + 1) * 2 * D],
                        et[:, g * 32:(g + 1) * 32],
                        slab_h[:, ql * D:(ql + 2) * D],
                        tile_position=(0, g * 32),
                    )
            cxs2 = []
            for p in range(2):
                cp = smp.tile([128, 2 * D], F16, tag=f"cxs{p}", name=f"cxs{p}")
                nc.vector.tensor_scalar_mul(
                    cp[:, :], cxp_all[:, p * 2 * D:(p + 1) * 2 * D], rpt_s[:, :]
                )
                cxs2.append(cp)

            if _STAGE < 5:
                continue
            # transpose each cxs chunk; copy only the useful columns
            # y = g*32 + j*8 + h into the packed ctxm_T layout.
            for p in range(2):
                for side in range(2):
                    j = 2 * p + side
                    for ic in range(2):
                        tp = tp_ps.tile([128, 128], F16, tag="tp_ps", name="tp_ps")
                        nc.tensor.transpose(
                            tp[:, :],
                            cxs2[p][:, side * D + ic * 128: side * D + (ic + 1) * 128],
                            ident_h[:, :],
                        )
                        src = tp.rearrange("p (g s) -> p g s", s=32)[
                            :, :, j * 8:(j + 1) * 8
                        ]
                        dst = cx_t[ic].rearrange(
                            "p (mm g s) -> p mm g s", mm=NMACRO, g=4
                        )[:, m, :, j * 8:(j + 1) * 8]
                        acopy(dst, src)


_NC = None


def _get_nc():
    global _NC
    if _NC is None:
        _NC = _build_program()
    return _NC


def _const_inputs():
    id32 = np.eye(128, dtype=np.float32)
    id16 = np.eye(128, dtype=np.float16)
    bsel = np.zeros((16, 128), dtype=np.float16)
    for q in range(16):
        bsel[q, q * 8:(q + 1) * 8] = 1
    ones = np.ones((128, 128), dtype=np.float16)
    return {"c_id32": id32, "c_id16": id16, "c_bsel": bsel, "c_ones": ones}


def kernel(**inputs):
    nc = _get_nc()
    full = {k: np.ascontiguousarray(v) for k, v in inputs.items()}
    consts = _const_inputs()
    in_maps = []
    for c in range(NCORES):
        sl = slice(c * BLOC, (c + 1) * BLOC)
        in_maps.append({
            "h": full["h"][sl],
            "h_arc": full["h_arc"][sl],
            "mask": full["mask"][sl],
            "Wq": full["Wq"], "Wk": full["Wk"], "Wv": full["Wv"], "Wo": full["Wo"],
            "bq": full["bq"], "bk": full["bk"], "bv": full["bv"], "bo": full["bo"],
            **consts,
        })
    res = run_bass_kernel_spmd(nc, in_maps, list(range(NCORES)))
    out = np.concatenate([res.results[c]["out"] for c in range(NCORES)], axis=0)
    return out.astype(np.float32)



# revision 3
# speedup vs baseline: 1.7503x; 1.7503x over previous
"""Trainium2 Bass kernel for nn_MultiHeadAttention_72765335929011.

Reference computation (per batch b, query q):
    q    = h @ Wq.T + bq                     (B, Lq, H*Dk)
    k    = h_arc @ Wk.T + bk                 (B, Lq, Lk, H*Dk)
    v    = h_arc @ Wv.T + bv
    s    = einsum('hd,khd->hk', q_, k_) / sqrt(D)
    attn = softmax(mask ? s : -1e9)
    ctx  = einsum('hk,khd->hd', attn, v)
    out  = ctx @ Wo.T + bo

Key restructure (exact algebra): with Dk=32 per head, fold Wk into the
query side and Wv@Wo into a per-head matrix F:
    qk[h,:]   = Wk_h.T @ q_h          -> scores = qk . h_arc  (i-contraction)
    ctxmT     = h_arc.T @ (attn/denom)     (k-contraction, model space)
    out       = sum_h ctxm_h @ F_h.T + (Wo@bv + bo),  F_h = Wo_h @ Wv_h
bk cancels in softmax (constant row shift); bv reduces to Wo@bv since
attn rows sum to 1. This cuts FLOPs ~15x and reads h_arc exactly once.

Host-side preprocessing (inside kernel(), before the device program):
  h_arc is cast to fp16 and laid out (B, Lk, Lq, D) so each macro-tile
  slab DMA is one 8KB-contiguous run per partition; the mask is folded
  to an additive fp16 bias lmask in {0, -800} on host.

Dataflow per macro-tile (16 queries, one (b,q)-slab of h_arc):
  slab fp16 (k x q*256+i) --PE transpose--> Tp (i-part) for scores;
  scoresT (k-part, q*8+h free) accumulates mask (lmask.T @ Bsel) plus
  per-q qk matmuls; exp on ACT gives eT; denominators via a ones-matmul
  partition reduction; normalization via a rank-1 R = ones x recip
  matmul and one DVE multiply (ets = eT * R); then ctxmT is computed
  DIRECTLY in the (i, q*8+h) transposed layout with per-(q,chunk)
  matmuls (slab chunk stationary, ets columns moving), so no transpose
  or scatter of the context is needed -- just two contiguous PSUM->SBUF
  copies per macro into the cx_t accumulator.

Sharding: data-parallel over B across 8 cores (2 batches each), no
collectives.  exp(scale*(s + lmask)) with scale=1/16 -> exp(s/16 - 50)
~ 2e-22 for masked slots (flushes to 0 in fp16); an all-masked row
normalizes to uniform exactly like the reference.
"""

import sys

import numpy as np

sys.path.insert(0, "/opt/trn_rl_repo")

import concourse.bass as bass
import concourse.mybir as mybir
import concourse.tile as tile
from concourse import bacc
from concourse.bass_utils import run_bass_kernel_spmd

F32 = mybir.dt.float32
F16 = mybir.dt.float16
I32 = mybir.dt.int32

B, LQ, LK, D = 16, 128, 128, 256
H, DKV = 8, 32
HD = H * DKV  # 256
NCORES = 8
BLOC = B // NCORES  # batches per core
N = BLOC * LQ  # 256 query rows per core
GQ = 16  # queries per macro-tile (GQ*H = 128)
NMACRO = N // GQ  # 16
MASK_RAW = -800.0
SCALE = 1.0 / 16.0  # 1/sqrt(D)


def _build_program(loop_reps=1):
    nc = bacc.Bacc(
        "TRN2",
        target_bir_lowering=False,
        debug=False,
        enable_asserts=True,
        num_devices=NCORES,
    )

    h_d = nc.dram_tensor("h", (BLOC, LQ, D), F32, kind="ExternalInput").ap()
    harc_d = nc.dram_tensor("h_arcT", (BLOC, LK, LQ, D), F16, kind="ExternalInput").ap()
    lmask_d = nc.dram_tensor("lmaskh", (BLOC, LQ, LK), F16, kind="ExternalInput").ap()
    wq_d = nc.dram_tensor("Wq", (HD, D), F32, kind="ExternalInput").ap()
    wk_d = nc.dram_tensor("Wk", (HD, D), F32, kind="ExternalInput").ap()
    wv_d = nc.dram_tensor("Wv", (HD, D), F32, kind="ExternalInput").ap()
    wo_d = nc.dram_tensor("Wo", (D, HD), F32, kind="ExternalInput").ap()
    bq_d = nc.dram_tensor("bq", (HD,), F32, kind="ExternalInput").ap()
    bv_d = nc.dram_tensor("bv", (HD,), F32, kind="ExternalInput").ap()
    bo_d = nc.dram_tensor("bo", (D,), F32, kind="ExternalInput").ap()
    # host-supplied constants (walrus codegen chokes on some affine_selects)
    id16_d = nc.dram_tensor("c_id16", (128, 128), F16, kind="ExternalInput").ap()
    bsel_d = nc.dram_tensor("c_bsel", (16, 128), F16, kind="ExternalInput").ap()
    ones_d = nc.dram_tensor("c_ones", (128, 128), F16, kind="ExternalInput").ap()
    out_d = nc.dram_tensor("out", (BLOC, LQ, D), F32, kind="ExternalOutput").ap()

    with tile.TileContext(nc) as tc:
        _emit(tc, h_d, harc_d, lmask_d, wq_d, wk_d, wv_d, wo_d, bq_d, bv_d, bo_d,
              id16_d, bsel_d, ones_d, out_d, loop_reps=loop_reps)

    nc.compile()
    return nc


def _emit(tc, h_d, harc_d, lmask_d, wq_d, wk_d, wv_d, wo_d, bq_d, bv_d, bo_d,
          id16_d, bsel_d, ones_d, out_d, loop_reps=1):
    import contextlib
    import os as _os
    _STAGE = int(_os.environ.get("KSTAGE", "9"))
    nc = tc.nc
    acopy = nc.any.tensor_copy

    with (
        tc.tile_pool(name="const", bufs=1) as constp,
        tc.tile_pool(name="wts", bufs=1) as wts,
        tc.tile_pool(name="big", bufs=1) as big,
        tc.tile_pool(name="slab", bufs=3) as slabp,
        tc.tile_pool(name="tpb", bufs=10) as tpbp,
        tc.tile_pool(name="sm", bufs=2) as smp,
        tc.tile_pool(name="tp_ps", bufs=3, space=bass.MemorySpace.PSUM) as tp_ps,
        tc.tile_pool(name="sc_ps", bufs=2, space=bass.MemorySpace.PSUM) as sc_ps,
        tc.tile_pool(name="r_ps", bufs=1, space=bass.MemorySpace.PSUM) as r_ps,
        tc.tile_pool(name="cx_ps", bufs=2, space=bass.MemorySpace.PSUM) as cx_ps,
    ):
        # ---------------- constants (DMA'd from host) ----------------
        ident_h = constp.tile([128, 128], F16, tag="ident_h", name="ident_h")
        nc.sync.dma_start(out=ident_h[:, :], in_=id16_d[:, :])

        # Bsel[q, y] = 1 iff y//8 == q  (16 x 128): lmask.T @ Bsel broadcasts
        # the per-(q,k) mask term to all 8 head columns of q.
        bsel = constp.tile([16, 128], F16, tag="bsel", name="bsel")
        nc.sync.dma_start(out=bsel[:, :], in_=bsel_d[:, :])

        onesf = constp.tile([128, 128], F16, tag="onesf", name="onesf")
        nc.sync.dma_start(out=onesf[:, :], in_=ones_d[:, :])
        ones1 = onesf[0:1, :]
        zbias = constp.tile([128, 1], F32, tag="zbias", name="zbias")
        nc.gpsimd.memset(zbias[:, :], 0.0)

        # whole-core additive mask, resident in SBUF: (16 q, m*128 + k)
        lmask_all = constp.tile([GQ, NMACRO * LK], F16, tag="lmask", name="lmask")
        nc.sync.dma_start(
            out=lmask_all.rearrange("q (b m k) -> q b m k", b=BLOC, m=LQ // GQ),
            in_=lmask_d.rearrange("b (m q) k -> q b m k", q=GQ),
        )

        # ---------------- weight / input staging (fp32 -> fp16) ----------------
        def load2h(name, dram):  # (256, 256) f32 -> two (128, 256) fp16 tiles
            out = []
            for c in range(2):
                t32 = wts.tile([128, D], F32, tag="stg32", name=f"{name}{c}_32")
                nc.sync.dma_start(out=t32[:, :], in_=dram[c * 128:(c + 1) * 128, :])
                t16 = wts.tile([128, D], F16, tag=f"{name}{c}", name=f"{name}{c}")
                acopy(t16[:, :], t32[:, :])
                out.append(t16)
            return out

        wq_s = load2h("wq", wq_d)
        wo_s = load2h("wo", wo_d)

        # Wk/Wv per-head at base partition 0: (32 d, h*256 + i), fp16.
        def loadph(name, dram):
            t32 = wts.tile([32, H * D], F32, tag="stgph", name=f"{name}_32")
            nc.sync.dma_start(
                out=t32.rearrange("d (h i) -> d h i", h=H),
                in_=dram.rearrange("(h d) i -> d h i", d=DKV),
            )
            t16 = wts.tile([32, H * D], F16, tag=name, name=name)
            acopy(t16[:, :], t32[:, :])
            return t16

        wk2 = loadph("wk2", wk_d)
        wv2 = loadph("wv2", wv_d)

        h_s = []
        for b in range(BLOC):
            t32 = wts.tile([128, D], F32, tag="stg32", name=f"h{b}_32")
            nc.sync.dma_start(out=t32[:, :], in_=h_d[b])
            t16 = wts.tile([128, D], F16, tag=f"h{b}", name=f"h{b}")
            acopy(t16[:, :], t32[:, :])
            h_s.append(t16)

        bq_s = constp.tile([32, H], F32, tag="bq", name="bq")
        nc.sync.dma_start(out=bq_s[:, :], in_=bq_d.rearrange("(h d) -> d h", d=DKV))
        bv32 = constp.tile([32, H], F32, tag="bv32", name="bv32")
        nc.sync.dma_start(out=bv32[:, :], in_=bv_d.rearrange("(h d) -> d h", d=DKV))
        bv_s = constp.tile([32, H], F16, tag="bv", name="bv")
        acopy(bv_s[:, :], bv32[:, :])
        bo_s = constp.tile([1, D], F32, tag="bo", name="bo")
        nc.sync.dma_start(out=bo_s[:, :], in_=bo_d.rearrange("(o d) -> o d", o=1))

        # fp16 transpose helper: (128,128) fp16 SBUF chunk -> fp16 SBUF dest
        def transpose_to(dst_ap, src_ap):
            tp = tp_ps.tile([128, 128], F16, tag="tp_ps", name="tp_ps")
            nc.tensor.transpose(tp[:, :], src_ap, ident_h[:, :])
            acopy(dst_ap, tp[:, :])

        # hT: (i 2x128, n=256);  WqT: (i 2x128, hd=256)   [fp16]
        h_t = [big.tile([128, N], F16, tag=f"ht{c}", name=f"ht{c}") for c in range(2)]
        for b in range(BLOC):
            for ic in range(2):
                transpose_to(
                    h_t[ic][:, b * 128:(b + 1) * 128],
                    h_s[b][:, ic * 128:(ic + 1) * 128],
                )
        wq_t = [big.tile([128, HD], F16, tag=f"wqt{c}", name=f"wqt{c}") for c in range(2)]
        for hc in range(2):
            for ic in range(2):
                transpose_to(
                    wq_t[ic][:, hc * 128:(hc + 1) * 128],
                    wq_s[hc][:, ic * 128:(ic + 1) * 128],
                )
        # WoT per-head at base 0: (32 d, h*256 + o), via 32-col strips.
        wo_t2 = big.tile([32, H * D], F16, tag="wot2", name="wot2")
        for oc in range(2):
            for hh in range(H):
                tp = tp_ps.tile([128, 128], F16, tag="tp_ps", name="tp_ps")
                nc.tensor.transpose(
                    tp[:32, :], wo_s[oc][:, hh * 32:(hh + 1) * 32], ident_h[:, :]
                )
                acopy(wo_t2[:, hh * D + oc * 128: hh * D + (oc + 1) * 128], tp[:32, :])

        # qT per head at base 0: (32 d, h*256 + n), fp16, +bq
        q_t2 = big.tile([32, H * N], F16, tag="qt2", name="qt2")
        for hh in range(H):
            qp = cx_ps.tile([32, N], F32, tag="cx", name="qp")
            for ic in range(2):
                nc.tensor.matmul(
                    qp[:, :],
                    wq_t[ic][:, hh * 32:(hh + 1) * 32],
                    h_t[ic][:, :],
                    start=(ic == 0), stop=(ic == 1),
                )
            nc.scalar.activation(
                q_t2[:, hh * N:(hh + 1) * N], qp[:, :],
                mybir.ActivationFunctionType.Identity,
                bias=bq_s[:, hh:hh + 1], scale=1.0,
            )

        # qkT[i, n*8+h] = sum_d Wk[h*32+d, i] * qT[h*32+d, n]   (fp16)
        qk_t = [big.tile([128, N * H], F16, tag=f"qkt{c}", name=f"qkt{c}")
                for c in range(2)]
        for hh in range(H):
            for ic in range(2):
                qkp = cx_ps.tile([128, N], F32, tag="cx", name="qkp")
                nc.tensor.matmul(
                    qkp[:, :],
                    wk2[:, hh * D + ic * 128: hh * D + (ic + 1) * 128],
                    q_t2[:, hh * N:(hh + 1) * N],
                )
                acopy(
                    qk_t[ic].rearrange("p (n h) -> p n h", h=H)[:, :, hh],
                    qkp[:, :],
                )

        # F_hT[i, o] = sum_d Wv[h*32+d, i] * Wo[o, h*32+d]  (fp16)
        f_t = [big.tile([128, H * D], F16, tag=f"ft{c}", name=f"ft{c}")
               for c in range(2)]
        for hh in range(H):
            for ic in range(2):
                fp = cx_ps.tile([128, D], F32, tag="cx", name="fp")
                nc.tensor.matmul(
                    fp[:, :],
                    wv2[:, hh * D + ic * 128: hh * D + (ic + 1) * 128],
                    wo_t2[:, hh * D:(hh + 1) * D],
                )
                acopy(f_t[ic][:, hh * D:(hh + 1) * D], fp[:, :])

        # bo_eff = Wo @ bv + bo  (1, 256) fp16
        bop = cx_ps.tile([1, D], F32, tag="cx", name="bop")
        for hh in range(H):
            nc.tensor.matmul(
                bop[:, :], bv_s[:, hh:hh + 1], wo_t2[:, hh * D:(hh + 1) * D],
                start=(hh == 0), stop=(hh == H - 1),
            )
        boe32 = constp.tile([1, D], F32, tag="boe32", name="boe32")
        nc.vector.tensor_add(boe32[:, :], bop[:, :], bo_s[:, :])
        boe = constp.tile([1, D], F16, tag="boe", name="boe")
        acopy(boe[:, :], boe32[:, :])

        # ctxm_T accumulator: (i 2x128, n*8+h), fp16
        cx_t = [big.tile([128, N * H], F16, tag=f"cxt{c}", name=f"cxt{c}")
                for c in range(2)]

        # ---------------- main loop over macro-tiles ----------------
        if _STAGE < 2:
            return
        loop_cm = tc.For_i(0, loop_reps, 1) if loop_reps > 1 else contextlib.nullcontext()
        with loop_cm:
            _emit_macros(
                tc, harc_d, _STAGE, acopy, slabp, smp, tpbp, tp_ps, sc_ps,
                r_ps, cx_ps, ident_h, bsel, onesf, zbias, lmask_all,
                qk_t, cx_t,
            )
        if _STAGE < 6:
            return
        # ---------------- output projection ----------------
        # out[n, o] = sum_h sum_i ctxm_T[i, n*8+h] * F_hT[i, o] + bo_eff
        for nck in range(BLOC):
            op = cx_ps.tile([128, D], F32, tag="cx", name="op")
            first = True
            for hh in range(H):
                for ic in range(2):
                    lhs = cx_t[ic].rearrange("p (n h) -> p n h", h=H)[
                        :, nck * 128:(nck + 1) * 128, hh
                    ]
                    nc.tensor.matmul(
                        op[:, :], lhs, f_t[ic][:, hh * D:(hh + 1) * D],
                        start=first, stop=False,
                    )
                    first = False
            nc.tensor.matmul(
                op[:, :], ones1[:, :], boe[:, :], start=False, stop=True,
            )
            os_ = smp.tile([128, D], F32, tag="os", name="os")
            nc.scalar.copy(os_[:, :], op[:, :])
            nc.sync.dma_start(out=out_d[nck], in_=os_[:, :])


def _emit_macros(tc, harc_d, _STAGE, acopy, slabp, smp, tpbp, tp_ps, sc_ps,
                 r_ps, cx_ps, ident_h, bsel, onesf, zbias, lmask_all,
                 qk_t, cx_t):
    nc = tc.nc
    ones_col = onesf[:, 0:1]
    for m in range(NMACRO):
        b, q0 = m // (LQ // GQ), (m % (LQ // GQ)) * GQ

        # one DMA: (128 k) partitions x one contiguous 8KB run (16q x 256i fp16)
        slab_h = slabp.tile([128, GQ * D], F16, tag="slab", name="slab")
        nc.sync.dma_start(
            out=slab_h.rearrange("k (q i) -> k q i", q=GQ),
            in_=harc_d[b, :, q0:q0 + GQ, :],
        )

        # scoresT (k-part, 16q*8h free); mask first:
        # (lmask.T @ Bsel)[k, q*8+h] = lmask[q, k]
        sct = sc_ps.tile([128, 128], F32, tag="sc_ps", name="sc_ps")
        nc.tensor.matmul(
            sct[:, :], lmask_all[:, m * LK:(m + 1) * LK], bsel[:, :],
            start=True, stop=False,
        )

        for ql in range(GQ):
            # both i-chunk transposes land in one PSUM tile -> one copy
            tpp = tp_ps.tile([128, 256], F16, tag="tp_ps", name="tp_ps")
            for ic in range(2):
                nc.tensor.transpose(
                    tpp[:, ic * 128:(ic + 1) * 128],
                    slab_h[:, ql * D + ic * 128: ql * D + (ic + 1) * 128],
                    ident_h[:, :],
                )
            tb = tpbp.tile([128, 256], F16, tag="tpb", name="tpb")
            # ~11/5 DVE/ACT split of PSUM->SBUF copies (ACT is ~2x slower)
            ceng = nc.scalar.copy if ql % 3 == 2 else nc.vector.tensor_copy
            ceng(tb[:, :], tpp[:, :])
            nq = m * GQ + ql
            for ic in range(2):
                nc.tensor.matmul(
                    sct[:, ql * 8:(ql + 1) * 8],
                    tb[:, ic * 128:(ic + 1) * 128],
                    qk_t[ic][:, nq * 8:(nq + 1) * 8],
                    start=False, stop=(ql == GQ - 1 and ic == 1),
                )

        if _STAGE < 3:
            continue
        # eT = exp(scoresT/16) stays in (k, qh) layout -> fp16
        et = smp.tile([128, 128], F16, tag="et", name="et")
        nc.scalar.activation(
            et[:, :], sct[:, :],
            mybir.ActivationFunctionType.Exp,
            bias=zbias[:, :], scale=SCALE,
        )
        # denom[qh] = sum_k eT[k, qh] into row 0 of the r tile; recip;
        # then R[k, qh] = recip[qh] via a rank-1 ones x recip matmul.
        r_t = r_ps.tile([128, 128], F32, tag="r", name="r")
        nc.tensor.matmul(r_t[0:1, :], ones_col[:, :], et[:, :])
        recip32 = smp.tile([1, 128], F32, tag="recip32", name="recip32")
        nc.vector.reciprocal(recip32[:, :], r_t[0:1, :])
        reciph = smp.tile([1, 128], F16, tag="reciph", name="reciph")
        nc.scalar.copy(reciph[:, :], recip32[:, :])
        nc.tensor.matmul(r_t[:, :], onesf[0:1, :], reciph[:, :])
        ets = smp.tile([128, 128], F16, tag="ets", name="ets")
        nc.vector.tensor_mul(ets[:, :], et[:, :], r_t[:, :])

        if _STAGE < 4:
            continue
        # ctxmT directly in packed (i, q*8+h) layout: per (q, i-chunk),
        # slab chunk stationary (k x 128i), ets columns moving (k x 8h)
        cxp = cx_ps.tile([128, 256], F32, tag="cx", name="cxp")
        for ql in range(GQ):
            for ic in range(2):
                nc.tensor.matmul(
                    cxp[:, ic * 128 + ql * 8: ic * 128 + (ql + 1) * 8],
                    slab_h[:, ql * D + ic * 128: ql * D + (ic + 1) * 128],
                    ets[:, ql * 8:(ql + 1) * 8],
                )

        if _STAGE < 5:
            continue
        for ic in range(2):
            ceng = nc.vector.tensor_copy if ic == 0 else nc.scalar.copy
            ceng(
                cx_t[ic][:, m * 128:(m + 1) * 128],
                cxp[:, ic * 128:(ic + 1) * 128],
            )


_NC = None


def _get_nc():
    global _NC
    if _NC is None:
        _NC = _build_program()
    return _NC


def _const_inputs():
    id16 = np.eye(128, dtype=np.float16)
    bsel = np.zeros((16, 128), dtype=np.float16)
    for q in range(16):
        bsel[q, q * 8:(q + 1) * 8] = 1
    ones = np.ones((128, 128), dtype=np.float16)
    return {"c_id16": id16, "c_bsel": bsel, "c_ones": ones}


def make_in_maps(full):
    """Host-side preprocessing + per-core sharding of the full inputs."""
    harc_t = np.ascontiguousarray(
        full["h_arc"].astype(np.float16).transpose(0, 2, 1, 3)
    )  # (B, Lk, Lq, D) fp16
    lmaskh = ((full["mask"].astype(np.float32) - 1.0) * (-MASK_RAW)).astype(
        np.float16
    )  # {0, MASK_RAW}
    consts = _const_inputs()
    in_maps = []
    for c in range(NCORES):
        sl = slice(c * BLOC, (c + 1) * BLOC)
        in_maps.append({
            "h": full["h"][sl],
            "h_arcT": harc_t[sl],
            "lmaskh": lmaskh[sl],
            "Wq": full["Wq"], "Wk": full["Wk"], "Wv": full["Wv"], "Wo": full["Wo"],
            "bq": full["bq"], "bv": full["bv"], "bo": full["bo"],
            **consts,
        })
    return in_maps


def kernel(**inputs):
    nc = _get_nc()
    full = {k: np.ascontiguousarray(v) for k, v in inputs.items()}
    in_maps = make_in_maps(full)
    res = run_bass_kernel_spmd(nc, in_maps, list(range(NCORES)))
    out = np.concatenate([res.results[c]["out"] for c in range(NCORES)], axis=0)
    return out.astype(np.float32)


# revision 31
# speedup vs baseline: 2.7122x; 1.5496x over previous
"""Trainium2 Bass kernel for nn_MultiHeadAttention_72765335929011.

Reference computation (per batch b, query q):
    q    = h @ Wq.T + bq                     (B, Lq, H*Dk)
    k    = h_arc @ Wk.T + bk                 (B, Lq, Lk, H*Dk)
    v    = h_arc @ Wv.T + bv
    s    = einsum('hd,khd->hk', q_, k_) / sqrt(D)
    attn = softmax(mask ? s : -1e9)
    ctx  = einsum('hk,khd->hd', attn, v)
    out  = ctx @ Wo.T + bo

Key restructure (exact algebra): with Dk=32 per head, fold Wk into the
query side and Wv@Wo into a per-head matrix F:
    qk[h,:]   = Wk_h.T @ q_h          -> scores = qk . h_arc  (i-contraction)
    ctxmT     = h_arc.T @ (attn/denom)     (k-contraction, model space)
    out       = sum_h ctxm_h @ F_h.T + (Wo@bv + bo),  F_h = Wo_h @ Wv_h
bk cancels in softmax (constant row shift); bv reduces to Wo@bv since
attn rows sum to 1. This cuts FLOPs ~15x and reads h_arc exactly once.

Host-side preprocessing (inside kernel(), before the device program):
  h_arc is cast to fp16 and laid out (B, Lk, Lq, D) so each macro-tile
  slab DMA is one 8KB-contiguous run per partition; the mask is folded
  to an additive fp16 bias lmask in {0, -800} on host.

Dataflow per macro-tile (16 queries, one (b,q)-slab of h_arc):
  slab fp16 (k x q*256+i) --PE transpose--> Tp (i-part) for scores;
  scoresT (k-part, q*8+h free) accumulates mask (lmask.T @ Bsel) plus
  per-q qk matmuls; exp on ACT gives eT; denominators via a ones-matmul
  partition reduction; normalization via a rank-1 R = ones x recip
  matmul and one DVE multiply (ets = eT * R); then ctxmT is computed
  DIRECTLY in the (i, q*8+h) transposed layout with per-(q,chunk)
  matmuls (slab chunk stationary, ets columns moving), so no transpose
  or scatter of the context is needed -- just two contiguous PSUM->SBUF
  copies per macro into the cx_t accumulator.

Software pipelining (emission order == scheduler priority): per macro m
the order is chain(m-1) [exp..ets, a full front of runway], then the
ctx block of macro m-2 (its LDWs absorb any DMA-wait at rep starts),
then front(m). The slab pool is 5 deep so the DMA queue runs several
macros ahead; the transposed-slab staging pool (tpb) is 16 deep.

Sharding: data-parallel over B across 8 cores (2 batches each), no
collectives.  exp(scale*(s + lmask)) with scale=1/16 -> exp(s/16 - 50)
~ 2e-22 for masked slots (flushes to 0 in fp16); an all-masked row
normalizes to uniform exactly like the reference.
"""

import sys

import numpy as np

sys.path.insert(0, "/opt/trn_rl_repo")

import concourse.bass as bass
import concourse.mybir as mybir
import concourse.tile as tile
from concourse import bacc
from concourse.bass_utils import run_bass_kernel_spmd

F32 = mybir.dt.float32
F16 = mybir.dt.float16
I32 = mybir.dt.int32

B, LQ, LK, D = 16, 128, 128, 256
H, DKV = 8, 32
HD = H * DKV  # 256
NCORES = 8
BLOC = B // NCORES  # batches per core
N = BLOC * LQ  # 256 query rows per core
GQ = 16  # queries per macro-tile (GQ*H = 128)
NMACRO = N // GQ  # 16
MASK_RAW = -800.0
SCALE = 1.0 / 16.0  # 1/sqrt(D)


def _build_program(loop_reps=1):
    nc = bacc.Bacc(
        "TRN2",
        target_bir_lowering=False,
        debug=False,
        enable_asserts=True,
        num_devices=NCORES,
    )

    h_d = nc.dram_tensor("h", (BLOC, LQ, D), F32, kind="ExternalInput").ap()
    harc_d = nc.dram_tensor("h_arcT", (BLOC, LK, LQ, D), F16, kind="ExternalInput").ap()
    lmask_d = nc.dram_tensor("lmaskh", (BLOC, LQ, LK), F16, kind="ExternalInput").ap()
    wq_d = nc.dram_tensor("Wq", (HD, D), F32, kind="ExternalInput").ap()
    wk_d = nc.dram_tensor("Wk", (HD, D), F32, kind="ExternalInput").ap()
    wv_d = nc.dram_tensor("Wv", (HD, D), F32, kind="ExternalInput").ap()
    wo_d = nc.dram_tensor("Wo", (D, HD), F32, kind="ExternalInput").ap()
    bq_d = nc.dram_tensor("bq", (HD,), F32, kind="ExternalInput").ap()
    bv_d = nc.dram_tensor("bv", (HD,), F32, kind="ExternalInput").ap()
    bo_d = nc.dram_tensor("bo", (D,), F32, kind="ExternalInput").ap()
    # host-supplied constants (walrus codegen chokes on some affine_selects)
    id16_d = nc.dram_tensor("c_id16", (128, 128), F16, kind="ExternalInput").ap()
    bsel_d = nc.dram_tensor("c_bsel", (16, 128), F16, kind="ExternalInput").ap()
    ones_d = nc.dram_tensor("c_ones", (128, 128), F16, kind="ExternalInput").ap()
    out_d = nc.dram_tensor("out", (BLOC, LQ, D), F32, kind="ExternalOutput").ap()

    with tile.TileContext(nc) as tc:
        _emit(tc, h_d, harc_d, lmask_d, wq_d, wk_d, wv_d, wo_d, bq_d, bv_d, bo_d,
              id16_d, bsel_d, ones_d, out_d, loop_reps=loop_reps)

    nc.compile()
    return nc


def _emit(tc, h_d, harc_d, lmask_d, wq_d, wk_d, wv_d, wo_d, bq_d, bv_d, bo_d,
          id16_d, bsel_d, ones_d, out_d, loop_reps=1):
    import contextlib
    import os as _os
    _STAGE = int(_os.environ.get("KSTAGE", "9"))
    nc = tc.nc
    acopy = nc.any.tensor_copy

    with (
        tc.tile_pool(name="const", bufs=1) as constp,
        tc.tile_pool(name="wts", bufs=1) as wts,
        tc.tile_pool(name="big", bufs=1) as big,
        tc.tile_pool(name="slab", bufs=5) as slabp,
        tc.tile_pool(name="tpb", bufs=16) as tpbp,
        tc.tile_pool(name="sm", bufs=3) as smp,
        tc.tile_pool(name="tp_ps", bufs=4, space=bass.MemorySpace.PSUM) as tp_ps,
        tc.tile_pool(name="sc_ps", bufs=2, space=bass.MemorySpace.PSUM) as sc_ps,
        tc.tile_pool(name="cx_ps", bufs=2, space=bass.MemorySpace.PSUM) as cx_ps,
    ):
        # ---------------- constants (DMA'd from host) ----------------
        ident_h = constp.tile([128, 128], F16, tag="ident_h", name="ident_h")
        nc.sync.dma_start(out=ident_h[:, :], in_=id16_d[:, :])

        # Bsel[q, y] = 1 iff y//8 == q  (16 x 128): lmask.T @ Bsel broadcasts
        # the per-(q,k) mask term to all 8 head columns of q.
        bsel = constp.tile([16, 128], F16, tag="bsel", name="bsel")
        nc.sync.dma_start(out=bsel[:, :], in_=bsel_d[:, :])

        onesf = constp.tile([128, 128], F16, tag="onesf", name="onesf")
        nc.sync.dma_start(out=onesf[:, :], in_=ones_d[:, :])
        ones1 = onesf[0:1, :]
        zbias = constp.tile([128, 1], F32, tag="zbias", name="zbias")
        nc.gpsimd.memset(zbias[:, :], 0.0)

        # whole-core additive mask, resident in SBUF: (16 q, m*128 + k)
        lmask_all = constp.tile([GQ, NMACRO * LK], F16, tag="lmask", name="lmask")
        nc.sync.dma_start(
            out=lmask_all.rearrange("q (b m k) -> q b m k", b=BLOC, m=LQ // GQ),
            in_=lmask_d.rearrange("b (m q) k -> q b m k", q=GQ),
        )

        # ---------------- weight / input staging (fp32 -> fp16) ----------------
        def load2h(name, dram):  # (256, 256) f32 -> two (128, 256) fp16 tiles
            out = []
            for c in range(2):
                t32 = wts.tile([128, D], F32, tag="stg32", name=f"{name}{c}_32")
                nc.sync.dma_start(out=t32[:, :], in_=dram[c * 128:(c + 1) * 128, :])
                t16 = wts.tile([128, D], F16, tag=f"{name}{c}", name=f"{name}{c}")
                acopy(t16[:, :], t32[:, :])
                out.append(t16)
            return out

        wq_s = load2h("wq", wq_d)
        wo_s = load2h("wo", wo_d)

        # Wk/Wv per-head at base partition 0: (32 d, h*256 + i), fp16.
        def loadph(name, dram):
            t32 = wts.tile([32, H * D], F32, tag="stgph", name=f"{name}_32")
            nc.sync.dma_start(
                out=t32.rearrange("d (h i) -> d h i", h=H),
                in_=dram.rearrange("(h d) i -> d h i", d=DKV),
            )
            t16 = wts.tile([32, H * D], F16, tag=name, name=name)
            acopy(t16[:, :], t32[:, :])
            return t16

        wk2 = loadph("wk2", wk_d)
        wv2 = loadph("wv2", wv_d)

        h_s = []
        for b in range(BLOC):
            t32 = wts.tile([128, D], F32, tag="stg32", name=f"h{b}_32")
            nc.sync.dma_start(out=t32[:, :], in_=h_d[b])
            t16 = wts.tile([128, D], F16, tag=f"h{b}", name=f"h{b}")
            acopy(t16[:, :], t32[:, :])
            h_s.append(t16)

        bq_s = constp.tile([32, H], F32, tag="bq", name="bq")
        nc.sync.dma_start(out=bq_s[:, :], in_=bq_d.rearrange("(h d) -> d h", d=DKV))
        bv32 = constp.tile([32, H], F32, tag="bv32", name="bv32")
        nc.sync.dma_start(out=bv32[:, :], in_=bv_d.rearrange("(h d) -> d h", d=DKV))
        bv_s = constp.tile([32, H], F16, tag="bv", name="bv")
        acopy(bv_s[:, :], bv32[:, :])
        bo_s = constp.tile([1, D], F32, tag="bo", name="bo")
        nc.sync.dma_start(out=bo_s[:, :], in_=bo_d.rearrange("(o d) -> o d", o=1))

        # fp16 transpose helper: (128,128) fp16 SBUF chunk -> fp16 SBUF dest
        def transpose_to(dst_ap, src_ap):
            tp = tp_ps.tile([128, 128], F16, tag="tp_ps", name="tp_ps")
            nc.tensor.transpose(tp[:, :], src_ap, ident_h[:, :])
            acopy(dst_ap, tp[:, :])

        # hT: (i 2x128, n=256);  WqT: (i 2x128, hd=256)   [fp16]
        h_t = [big.tile([128, N], F16, tag=f"ht{c}", name=f"ht{c}") for c in range(2)]
        for b in range(BLOC):
            for ic in range(2):
                transpose_to(
                    h_t[ic][:, b * 128:(b + 1) * 128],
                    h_s[b][:, ic * 128:(ic + 1) * 128],
                )
        wq_t = [big.tile([128, HD], F16, tag=f"wqt{c}", name=f"wqt{c}") for c in range(2)]
        for hc in range(2):
            for ic in range(2):
                transpose_to(
                    wq_t[ic][:, hc * 128:(hc + 1) * 128],
                    wq_s[hc][:, ic * 128:(ic + 1) * 128],
                )
        # WoT per-head at base 0: (32 d, h*256 + o), via 32-col strips.
        wo_t2 = big.tile([32, H * D], F16, tag="wot2", name="wot2")
        for oc in range(2):
            for hh in range(H):
                tp = tp_ps.tile([128, 128], F16, tag="tp_ps", name="tp_ps")
                nc.tensor.transpose(
                    tp[:32, :], wo_s[oc][:, hh * 32:(hh + 1) * 32], ident_h[:, :]
                )
                acopy(wo_t2[:, hh * D + oc * 128: hh * D + (oc + 1) * 128], tp[:32, :])

        # qT per head at base 0: (32 d, h*256 + n), fp16, +bq
        q_t2 = big.tile([32, H * N], F16, tag="qt2", name="qt2")
        for hh in range(H):
            qp = cx_ps.tile([32, N], F32, tag="cx", name="qp")
            for ic in range(2):
                nc.tensor.matmul(
                    qp[:, :],
                    wq_t[ic][:, hh * 32:(hh + 1) * 32],
                    h_t[ic][:, :],
                    start=(ic == 0), stop=(ic == 1),
                )
            nc.scalar.activation(
                q_t2[:, hh * N:(hh + 1) * N], qp[:, :],
                mybir.ActivationFunctionType.Identity,
                bias=bq_s[:, hh:hh + 1], scale=1.0,
            )

        # qkT[i, n*8+h] = sum_d Wk[h*32+d, i] * qT[h*32+d, n]   (fp16)
        qk_t = [big.tile([128, N * H], F16, tag=f"qkt{c}", name=f"qkt{c}")
                for c in range(2)]
        for hh in range(H):
            for ic in range(2):
                qkp = cx_ps.tile([128, N], F32, tag="cx", name="qkp")
                nc.tensor.matmul(
                    qkp[:, :],
                    wk2[:, hh * D + ic * 128: hh * D + (ic + 1) * 128],
                    q_t2[:, hh * N:(hh + 1) * N],
                )
                acopy(
                    qk_t[ic].rearrange("p (n h) -> p n h", h=H)[:, :, hh],
                    qkp[:, :],
                )

        # F_hT[i, o] = sum_d Wv[h*32+d, i] * Wo[o, h*32+d]  (fp16)
        f_t = [big.tile([128, H * D], F16, tag=f"ft{c}", name=f"ft{c}")
               for c in range(2)]
        for hh in range(H):
            for ic in range(2):
                fp = cx_ps.tile([128, D], F32, tag="cx", name="fp")
                nc.tensor.matmul(
                    fp[:, :],
                    wv2[:, hh * D + ic * 128: hh * D + (ic + 1) * 128],
                    wo_t2[:, hh * D:(hh + 1) * D],
                )
                acopy(f_t[ic][:, hh * D:(hh + 1) * D], fp[:, :])

        # bo_eff = Wo @ bv + bo  (1, 256) fp16
        bop = cx_ps.tile([1, D], F32, tag="cx", name="bop")
        for hh in range(H):
            nc.tensor.matmul(
                bop[:, :], bv_s[:, hh:hh + 1], wo_t2[:, hh * D:(hh + 1) * D],
                start=(hh == 0), stop=(hh == H - 1),
            )
        boe32 = constp.tile([1, D], F32, tag="boe32", name="boe32")
        nc.vector.tensor_add(boe32[:, :], bop[:, :], bo_s[:, :])
        boe = constp.tile([1, D], F16, tag="boe", name="boe")
        acopy(boe[:, :], boe32[:, :])

        # ctxm_T accumulator: (i 2x128, n*8+h), fp16
        cx_t = [big.tile([128, N * H], F16, tag=f"cxt{c}", name=f"cxt{c}")
                for c in range(2)]

        # ---------------- main loop over macro-tiles ----------------
        if _STAGE < 2:
            return
        loop_cm = tc.For_i(0, loop_reps, 1) if loop_reps > 1 else contextlib.nullcontext()
        nbody = int(_os.environ.get("KBODY", "1"))
        with loop_cm:
            for _ in range(nbody):
                _emit_macros(
                    tc, harc_d, _STAGE, acopy, slabp, smp, tpbp, tp_ps, sc_ps,
                    cx_ps, ident_h, bsel, onesf, zbias, lmask_all,
                    qk_t, cx_t,
                )
        if _STAGE < 6:
            return
        # ---------------- output projection ----------------
        # out[n, o] = sum_h sum_i ctxm_T[i, n*8+h] * F_hT[i, o] + bo_eff
        for nck in range(BLOC):
            op = cx_ps.tile([128, D], F32, tag="cx", name="op")
            first = True
            for hh in range(H):
                for ic in range(2):
                    lhs = cx_t[ic].rearrange("p (n h) -> p n h", h=H)[
                        :, nck * 128:(nck + 1) * 128, hh
                    ]
                    nc.tensor.matmul(
                        op[:, :], lhs, f_t[ic][:, hh * D:(hh + 1) * D],
                        start=first, stop=False,
                    )
                    first = False
            nc.tensor.matmul(
                op[:, :], ones1[:, :], boe[:, :], start=False, stop=True,
            )
            os_ = smp.tile([128, D], F32, tag="os", name="os")
            nc.scalar.copy(os_[:, :], op[:, :])
            nc.sync.dma_start(out=out_d[nck], in_=os_[:, :])


def _emit_macros(tc, harc_d, _STAGE, acopy, slabp, smp, tpbp, tp_ps, sc_ps,
                 cx_ps, ident_h, bsel, onesf, zbias, lmask_all,
                 qk_t, cx_t):
    nc = tc.nc
    ones_col = onesf[:, 0:1]
    state = {}

    def chain(m):
        # softmax chain: exp -> denom -> recip -> rank-1 R -> ets
        slab_h, sct = state[m]
        et = smp.tile([128, 128], F16, tag="et", name="et")
        nc.scalar.activation(
            et[:, :], sct[:, :],
            mybir.ActivationFunctionType.Exp,
            bias=zbias[:, :], scale=SCALE,
        )
        rcx = cx_ps.tile([128, 384], F32, tag="cx", name="rcx")
        r_t = rcx[:, 256:384]
        cxp = rcx[:, 0:256]
        nc.tensor.matmul(r_t[0:1, :], ones_col[:, :], et[:, :])
        reciph = smp.tile([1, 128], F16, tag="reciph", name="reciph")
        with nc.allow_low_precision(reason="softmax recip fits fp16"):
            nc.vector.reciprocal(reciph[:, :], r_t[0:1, :])
        nc.tensor.matmul(r_t[:, :], onesf[0:1, :], reciph[:, :])
        ets = smp.tile([128, 128], F16, tag="ets", name="ets")
        nc.vector.tensor_mul(ets[:, :], et[:, :], r_t[:, :])
        state[m] = (slab_h, ets, cxp)

    def ctx_pairs(m, qls):
        # ctxmT in packed (i, q*8+h) layout: slab chunk stationary (k x 128i),
        # ets columns moving (k x 8h). Emitted interleaved into the NEXT
        # macro's transpose stream so these LDWs hide under its 128-cycle
        # identity streams.
        if _STAGE < 4:
            return
        slab_h, ets, cxp = state[m]
        for ql in qls:
            for ic in range(2):
                nc.tensor.matmul(
                    cxp[:, ic * 128 + ql * 8: ic * 128 + (ql + 1) * 8],
                    slab_h[:, ql * D + ic * 128: ql * D + (ic + 1) * 128],
                    ets[:, ql * 8:(ql + 1) * 8],
                )

    def ctx_flush(m):
        slab_h, ets, cxp = state.pop(m)
        if _STAGE < 5:
            return
        for ic in range(2):
            ceng = nc.vector.tensor_copy if ic == 0 else nc.scalar.copy
            ceng(
                cx_t[ic][:, m * 128:(m + 1) * 128],
                cxp[:, ic * 128:(ic + 1) * 128],
            )

    def front(m, prev):
        b, q0 = m // (LQ // GQ), (m % (LQ // GQ)) * GQ

        # one DMA: (128 k) partitions x one contiguous 8KB run (16q x 256i fp16)
        slab_h = slabp.tile([128, GQ * D], F16, tag="slab", name="slab")
        nc.sync.dma_start(
            out=slab_h.rearrange("k (q i) -> k q i", q=GQ),
            in_=harc_d[b, :, q0:q0 + GQ, :],
        )

        # scoresT (k-part, 16q*8h free); mask first:
        # (lmask.T @ Bsel)[k, q*8+h] = lmask[q, k]
        sct = sc_ps.tile([128, 128], F32, tag="sc_ps", name="sc_ps")
        nc.tensor.matmul(
            sct[:, :], lmask_all[:, m * LK:(m + 1) * LK], bsel[:, :],
            start=True, stop=False,
        )

        for p2 in range(GQ // 2):
            # two queries' transposes (4 chunks) land in one PSUM tile ->
            # one (128,512) copy; fewer ops amortize the per-op overhead
            tpp = tp_ps.tile([128, 512], F16, tag="tp_ps", name="tp_ps")
            for j in range(2):
                ql = p2 * 2 + j
                for ic in range(2):
                    nc.tensor.transpose(
                        tpp[:, j * 256 + ic * 128: j * 256 + (ic + 1) * 128],
                        slab_h[:, ql * D + ic * 128: ql * D + (ic + 1) * 128],
                        ident_h[:, :],
                    )
            tb = tpbp.tile([128, 512], F16, tag="tpb", name="tpb")
            # 5/3 DVE/ACT split of PSUM->SBUF copies (ACT is ~1.5x slower)
            ceng = nc.scalar.copy if p2 in (3, 6) else nc.vector.tensor_copy
            ceng(tb[:, :], tpp[:, :])
            for j in range(2):
                ql = p2 * 2 + j
                nq = m * GQ + ql
                for ic in range(2):
                    nc.tensor.matmul(
                        sct[:, ql * 8:(ql + 1) * 8],
                        tb[:, j * 256 + ic * 128: j * 256 + (ic + 1) * 128],
                        qk_t[ic][:, nq * 8:(nq + 1) * 8],
                        start=False, stop=(ql == GQ - 1 and ic == 1),
                    )
        state[m] = (slab_h, sct)

    # software-pipelined emission, tail depth 2: macro m's softmax chain is
    # emitted before front(m+1) (a full front of runway), and m's context
    # matmuls are interleaved into front(m+2)'s transpose stream so their
    # LDWs hide under the 128-cycle identity streams.
    for m in range(NMACRO):
        if m >= 1 and _STAGE >= 3:
            chain(m - 1)
        if m >= 2 and _STAGE >= 3:
            ctx_pairs(m - 2, range(GQ))
            ctx_flush(m - 2)
        front(m, None)
    if _STAGE >= 3:
        chain(NMACRO - 1)
        for mm in (NMACRO - 2, NMACRO - 1):
            ctx_pairs(mm, range(GQ))
            ctx_flush(mm)


_NC = None


def _get_nc():
    global _NC
    if _NC is None:
        _NC = _build_program()
    return _NC


def _const_inputs():
    id16 = np.eye(128, dtype=np.float16)
    bsel = np.zeros((16, 128), dtype=np.float16)
    for q in range(16):
        bsel[q, q * 8:(q + 1) * 8] = 1
    ones = np.ones((128, 128), dtype=np.float16)
    return {"c_id16": id16, "c_bsel": bsel, "c_ones": ones}


def make_in_maps(full):
    """Host-side preprocessing + per-core sharding of the full inputs."""
    harc_t = np.ascontiguousarray(
        full["h_arc"].astype(np.float16).transpose(0, 2, 1, 3)
    )  # (B, Lk, Lq, D) fp16
    lmaskh = ((full["mask"].astype(np.float32) - 1.0) * (-MASK_RAW)).astype(
        np.float16
    )  # {0, MASK_RAW}
    consts = _const_inputs()
    in_maps = []
    for c in range(NCORES):
        sl = slice(c * BLOC, (c + 1) * BLOC)
        in_maps.append({
            "h": full["h"][sl],
            "h_arcT": harc_t[sl],
            "lmaskh": lmaskh[sl],
            "Wq": full["Wq"], "Wk": full["Wk"], "Wv": full["Wv"], "Wo": full["Wo"],
            "bq": full["bq"], "bv": full["bv"], "bo": full["bo"],
            **consts,
        })
    return in_maps


def kernel(**inputs):
    nc = _get_nc()
    full = {k: np.ascontiguousarray(v) for k, v in inputs.items()}
    in_maps = make_in_maps(full)
    res = run_bass_kernel_spmd(nc, in_maps, list(range(NCORES)))
    out = np.concatenate([res.results[c]["out"] for c in range(NCORES)], axis=0)
    return out.astype(np.float32)


# revision 33
# speedup vs baseline: 2.9272x; 1.0793x over previous
"""Trainium2 Bass kernel for nn_MultiHeadAttention_72765335929011.

Reference computation (per batch b, query q):
    q    = h @ Wq.T + bq                     (B, Lq, H*Dk)
    k    = h_arc @ Wk.T + bk                 (B, Lq, Lk, H*Dk)
    v    = h_arc @ Wv.T + bv
    s    = einsum('hd,khd->hk', q_, k_) / sqrt(D)
    attn = softmax(mask ? s : -1e9)
    ctx  = einsum('hk,khd->hd', attn, v)
    out  = ctx @ Wo.T + bo

Key restructure (exact algebra): with Dk=32 per head, fold Wk into the
query side and Wv@Wo into a per-head matrix F:
    qk[h,:]   = Wk_h.T @ q_h          -> scores = qk . h_arc  (i-contraction)
    ctxmT     = h_arc.T @ (attn/denom)     (k-contraction, model space)
    out       = sum_h ctxm_h @ F_h.T + (Wo@bv + bo),  F_h = Wo_h @ Wv_h
bk cancels in softmax (constant row shift); bv reduces to Wo@bv since
attn rows sum to 1. This cuts FLOPs ~15x and reads h_arc exactly once.

Host-side preprocessing (inside kernel(), before the device program):
  h_arc is cast to fp16 and laid out (B, Lk, Lq, D) so each macro-tile
  slab DMA is one 8KB-contiguous run per partition; the mask is folded
  to an additive fp16 bias lmask in {0, -800} on host.

Dataflow per macro-tile (16 queries, one (b,q)-slab of h_arc):
  slab fp16 (k x q*256+i) --PE transpose--> Tp (i-part) for scores;
  scoresT (k-part, q*8+h free) accumulates mask (lmask.T @ Bsel) plus
  per-q qk matmuls; exp on ACT gives eT; denominators via a ones-matmul
  partition reduction; normalization via a rank-1 R = ones x recip
  matmul and one DVE multiply (ets = eT * R); then ctxmT is computed
  DIRECTLY in the (i, q*8+h) transposed layout with per-(q,chunk)
  matmuls (slab chunk stationary, ets columns moving), so no transpose
  or scatter of the context is needed -- just two contiguous PSUM->SBUF
  copies per macro into the cx_t accumulator.

Software pipelining (emission order == scheduler priority): per macro m
the order is chain(m-1) [exp..ets, a full front of runway], then the
ctx block of macro m-2 (its LDWs absorb any DMA-wait at rep starts),
then front(m). The slab pool is 5 deep so the DMA queue runs several
macros ahead; the transposed-slab staging pool (tpb) is 16 deep.

Sharding: data-parallel over B across 8 cores (2 batches each), no
collectives.  exp(scale*(s + lmask)) with scale=1/16 -> exp(s/16 - 50)
~ 2e-22 for masked slots (flushes to 0 in fp16); an all-masked row
normalizes to uniform exactly like the reference.
"""

import sys

import numpy as np

sys.path.insert(0, "/opt/trn_rl_repo")

import concourse.bass as bass
import concourse.mybir as mybir
import concourse.tile as tile
from concourse import bacc
from concourse.bass_utils import run_bass_kernel_spmd

F32 = mybir.dt.float32
F16 = mybir.dt.float16
I32 = mybir.dt.int32

B, LQ, LK, D = 16, 128, 128, 256
H, DKV = 8, 32
HD = H * DKV  # 256
NCORES = 8
BLOC = B // NCORES  # batches per core
N = BLOC * LQ  # 256 query rows per core
GQ = 16  # queries per macro-tile (GQ*H = 128)
NMACRO = N // GQ  # 16
MASK_RAW = -800.0
SCALE = 1.0 / 16.0  # 1/sqrt(D)


def _build_program(loop_reps=1):
    nc = bacc.Bacc(
        "TRN2",
        target_bir_lowering=False,
        debug=False,
        enable_asserts=True,
        num_devices=NCORES,
    )

    h_d = nc.dram_tensor("h", (BLOC, LQ, D), F32, kind="ExternalInput").ap()
    harc_d = nc.dram_tensor("h_arcT", (BLOC, LK, LQ, D), F16, kind="ExternalInput").ap()
    lmask_d = nc.dram_tensor("lmaskh", (BLOC, LQ, LK), F16, kind="ExternalInput").ap()
    wq_d = nc.dram_tensor("Wq", (HD, D), F32, kind="ExternalInput").ap()
    wk_d = nc.dram_tensor("Wk", (HD, D), F32, kind="ExternalInput").ap()
    wv_d = nc.dram_tensor("Wv", (HD, D), F32, kind="ExternalInput").ap()
    wo_d = nc.dram_tensor("Wo", (D, HD), F32, kind="ExternalInput").ap()
    bq_d = nc.dram_tensor("bq", (HD,), F32, kind="ExternalInput").ap()
    bv_d = nc.dram_tensor("bv", (HD,), F32, kind="ExternalInput").ap()
    bo_d = nc.dram_tensor("bo", (D,), F32, kind="ExternalInput").ap()
    # host-supplied constants (walrus codegen chokes on some affine_selects)
    id16_d = nc.dram_tensor("c_id16", (128, 128), F16, kind="ExternalInput").ap()
    bsel_d = nc.dram_tensor("c_bsel", (16, 128), F16, kind="ExternalInput").ap()
    ones_d = nc.dram_tensor("c_ones", (128, 128), F16, kind="ExternalInput").ap()
    out_d = nc.dram_tensor("out", (BLOC, LQ, D), F32, kind="ExternalOutput").ap()

    with tile.TileContext(nc) as tc:
        _emit(tc, h_d, harc_d, lmask_d, wq_d, wk_d, wv_d, wo_d, bq_d, bv_d, bo_d,
              id16_d, bsel_d, ones_d, out_d, loop_reps=loop_reps)

    nc.compile()
    return nc


def _emit(tc, h_d, harc_d, lmask_d, wq_d, wk_d, wv_d, wo_d, bq_d, bv_d, bo_d,
          id16_d, bsel_d, ones_d, out_d, loop_reps=1):
    import contextlib
    import os as _os
    _STAGE = int(_os.environ.get("KSTAGE", "9"))
    nc = tc.nc
    acopy = nc.any.tensor_copy

    with (
        tc.tile_pool(name="const", bufs=1) as constp,
        tc.tile_pool(name="wts", bufs=1) as wts,
        tc.tile_pool(name="big", bufs=1) as big,
        tc.tile_pool(name="slab", bufs=5) as slabp,
        tc.tile_pool(name="tpb", bufs=16) as tpbp,
        tc.tile_pool(name="sm", bufs=3) as smp,
        tc.tile_pool(name="tp_ps", bufs=4, space=bass.MemorySpace.PSUM) as tp_ps,
        tc.tile_pool(name="sc_ps", bufs=2, space=bass.MemorySpace.PSUM) as sc_ps,
        tc.tile_pool(name="cx_ps", bufs=2, space=bass.MemorySpace.PSUM) as cx_ps,
    ):
        # ---------------- constants (DMA'd from host) ----------------
        ident_h = constp.tile([128, 128], F16, tag="ident_h", name="ident_h")
        nc.sync.dma_start(out=ident_h[:, :], in_=id16_d[:, :])

        # Bsel[q, y] = 1 iff y//8 == q  (16 x 128): lmask.T @ Bsel broadcasts
        # the per-(q,k) mask term to all 8 head columns of q.
        bsel = constp.tile([16, 128], F16, tag="bsel", name="bsel")
        nc.sync.dma_start(out=bsel[:, :], in_=bsel_d[:, :])

        onesf = constp.tile([128, 128], F16, tag="onesf", name="onesf")
        nc.sync.dma_start(out=onesf[:, :], in_=ones_d[:, :])
        ones1 = onesf[0:1, :]
        zbias = constp.tile([128, 1], F32, tag="zbias", name="zbias")
        nc.gpsimd.memset(zbias[:, :], 0.0)

        # whole-core additive mask, resident in SBUF: (16 q, m*128 + k)
        lmask_all = constp.tile([GQ, NMACRO * LK], F16, tag="lmask", name="lmask")
        nc.sync.dma_start(
            out=lmask_all.rearrange("q (b m k) -> q b m k", b=BLOC, m=LQ // GQ),
            in_=lmask_d.rearrange("b (m q) k -> q b m k", q=GQ),
        )

        # ---------------- weight / input staging (fp32 -> fp16) ----------------
        def load2h(name, dram):  # (256, 256) f32 -> two (128, 256) fp16 tiles
            out = []
            for c in range(2):
                t32 = wts.tile([128, D], F32, tag="stg32", name=f"{name}{c}_32")
                nc.sync.dma_start(out=t32[:, :], in_=dram[c * 128:(c + 1) * 128, :])
                t16 = wts.tile([128, D], F16, tag=f"{name}{c}", name=f"{name}{c}")
                acopy(t16[:, :], t32[:, :])
                out.append(t16)
            return out

        wq_s = load2h("wq", wq_d)
        wo_s = load2h("wo", wo_d)

        # Wk/Wv per-head at base partition 0: (32 d, h*256 + i), fp16.
        def loadph(name, dram):
            t32 = wts.tile([32, H * D], F32, tag="stgph", name=f"{name}_32")
            nc.sync.dma_start(
                out=t32.rearrange("d (h i) -> d h i", h=H),
                in_=dram.rearrange("(h d) i -> d h i", d=DKV),
            )
            t16 = wts.tile([32, H * D], F16, tag=name, name=name)
            acopy(t16[:, :], t32[:, :])
            return t16

        wk2 = loadph("wk2", wk_d)
        wv2 = loadph("wv2", wv_d)

        h_s = []
        for b in range(BLOC):
            t32 = wts.tile([128, D], F32, tag="stg32", name=f"h{b}_32")
            nc.sync.dma_start(out=t32[:, :], in_=h_d[b])
            t16 = wts.tile([128, D], F16, tag=f"h{b}", name=f"h{b}")
            acopy(t16[:, :], t32[:, :])
            h_s.append(t16)

        bq_s = constp.tile([32, H], F32, tag="bq", name="bq")
        nc.sync.dma_start(out=bq_s[:, :], in_=bq_d.rearrange("(h d) -> d h", d=DKV))
        bv32 = constp.tile([32, H], F32, tag="bv32", name="bv32")
        nc.sync.dma_start(out=bv32[:, :], in_=bv_d.rearrange("(h d) -> d h", d=DKV))
        bv_s = constp.tile([32, H], F16, tag="bv", name="bv")
        acopy(bv_s[:, :], bv32[:, :])
        bo_s = constp.tile([1, D], F32, tag="bo", name="bo")
        nc.sync.dma_start(out=bo_s[:, :], in_=bo_d.rearrange("(o d) -> o d", o=1))

        # fp16 transpose helper: (128,128) fp16 SBUF chunk -> fp16 SBUF dest
        def transpose_to(dst_ap, src_ap):
            tp = tp_ps.tile([128, 128], F16, tag="tp_ps", name="tp_ps")
            nc.tensor.transpose(tp[:, :], src_ap, ident_h[:, :])
            acopy(dst_ap, tp[:, :])

        # hT: (i 2x128, n=256);  WqT: (i 2x128, hd=256)   [fp16]
        h_t = [big.tile([128, N], F16, tag=f"ht{c}", name=f"ht{c}") for c in range(2)]
        for b in range(BLOC):
            for ic in range(2):
                transpose_to(
                    h_t[ic][:, b * 128:(b + 1) * 128],
                    h_s[b][:, ic * 128:(ic + 1) * 128],
                )
        wq_t = [big.tile([128, HD], F16, tag=f"wqt{c}", name=f"wqt{c}") for c in range(2)]
        for hc in range(2):
            for ic in range(2):
                transpose_to(
                    wq_t[ic][:, hc * 128:(hc + 1) * 128],
                    wq_s[hc][:, ic * 128:(ic + 1) * 128],
                )
        # WoT per-head at base 0: (32 d, h*256 + o), via 32-col strips.
        wo_t2 = big.tile([32, H * D], F16, tag="wot2", name="wot2")
        for oc in range(2):
            for hh in range(H):
                tp = tp_ps.tile([128, 128], F16, tag="tp_ps", name="tp_ps")
                nc.tensor.transpose(
                    tp[:32, :], wo_s[oc][:, hh * 32:(hh + 1) * 32], ident_h[:, :]
                )
                acopy(wo_t2[:, hh * D + oc * 128: hh * D + (oc + 1) * 128], tp[:32, :])

        # qT per head at base 0: (32 d, h*256 + n), fp16, +bq
        q_t2 = big.tile([32, H * N], F16, tag="qt2", name="qt2")
        for hh in range(H):
            qp = cx_ps.tile([32, N], F32, tag="cx", name="qp")
            for ic in range(2):
                nc.tensor.matmul(
                    qp[:, :],
                    wq_t[ic][:, hh * 32:(hh + 1) * 32],
                    h_t[ic][:, :],
                    start=(ic == 0), stop=(ic == 1),
                )
            nc.scalar.activation(
                q_t2[:, hh * N:(hh + 1) * N], qp[:, :],
                mybir.ActivationFunctionType.Identity,
                bias=bq_s[:, hh:hh + 1], scale=1.0,
            )

        # qkT[i, n*8+h] = sum_d Wk[h*32+d, i] * qT[h*32+d, n]   (fp16)
        qk_t = [big.tile([128, N * H], F16, tag=f"qkt{c}", name=f"qkt{c}")
                for c in range(2)]
        for hh in range(H):
            for ic in range(2):
                qkp = cx_ps.tile([128, N], F32, tag="cx", name="qkp")
                nc.tensor.matmul(
                    qkp[:, :],
                    wk2[:, hh * D + ic * 128: hh * D + (ic + 1) * 128],
                    q_t2[:, hh * N:(hh + 1) * N],
                )
                acopy(
                    qk_t[ic].rearrange("p (n h) -> p n h", h=H)[:, :, hh],
                    qkp[:, :],
                )

        # F_hT[i, o] = sum_d Wv[h*32+d, i] * Wo[o, h*32+d]  (fp16)
        f_t = [big.tile([128, H * D], F16, tag=f"ft{c}", name=f"ft{c}")
               for c in range(2)]
        for hh in range(H):
            for ic in range(2):
                fp = cx_ps.tile([128, D], F32, tag="cx", name="fp")
                nc.tensor.matmul(
                    fp[:, :],
                    wv2[:, hh * D + ic * 128: hh * D + (ic + 1) * 128],
                    wo_t2[:, hh * D:(hh + 1) * D],
                )
                acopy(f_t[ic][:, hh * D:(hh + 1) * D], fp[:, :])

        # bo_eff = Wo @ bv + bo  (1, 256) fp16
        bop = cx_ps.tile([1, D], F32, tag="cx", name="bop")
        for hh in range(H):
            nc.tensor.matmul(
                bop[:, :], bv_s[:, hh:hh + 1], wo_t2[:, hh * D:(hh + 1) * D],
                start=(hh == 0), stop=(hh == H - 1),
            )
        boe32 = constp.tile([1, D], F32, tag="boe32", name="boe32")
        nc.vector.tensor_add(boe32[:, :], bop[:, :], bo_s[:, :])
        boe = constp.tile([1, D], F16, tag="boe", name="boe")
        acopy(boe[:, :], boe32[:, :])

        # ctxm_T accumulator: (i 2x128, n*8+h), fp16
        cx_t = [big.tile([128, N * H], F16, tag=f"cxt{c}", name=f"cxt{c}")
                for c in range(2)]

        # ---------------- main loop over macro-tiles ----------------
        if _STAGE < 2:
            return
        loop_cm = tc.For_i(0, loop_reps, 1) if loop_reps > 1 else contextlib.nullcontext()
        with loop_cm:
            _emit_macros(
                    tc, harc_d, _STAGE, acopy, slabp, smp, tpbp, tp_ps, sc_ps,
                    cx_ps, ident_h, bsel, onesf, zbias, lmask_all,
                    qk_t, cx_t,
                )
        if _STAGE < 6:
            return
        # ---------------- output projection ----------------
        # out[n, o] = sum_h sum_i ctxm_T[i, n*8+h] * F_hT[i, o] + bo_eff
        for nck in range(BLOC):
            op = cx_ps.tile([128, D], F32, tag="cx", name="op")
            first = True
            for hh in range(H):
                for ic in range(2):
                    lhs = cx_t[ic].rearrange("p (n h) -> p n h", h=H)[
                        :, nck * 128:(nck + 1) * 128, hh
                    ]
                    nc.tensor.matmul(
                        op[:, :], lhs, f_t[ic][:, hh * D:(hh + 1) * D],
                        start=first, stop=False,
                    )
                    first = False
            nc.tensor.matmul(
                op[:, :], ones1[:, :], boe[:, :], start=False, stop=True,
            )
            os_ = smp.tile([128, D], F32, tag="os", name="os")
            nc.scalar.copy(os_[:, :], op[:, :])
            nc.sync.dma_start(out=out_d[nck], in_=os_[:, :])


def _emit_macros(tc, harc_d, _STAGE, acopy, slabp, smp, tpbp, tp_ps, sc_ps,
                 cx_ps, ident_h, bsel, onesf, zbias, lmask_all,
                 qk_t, cx_t):
    nc = tc.nc
    ones_col = onesf[:, 0:1]
    state = {}

    def chain(m):
        # softmax chain: exp -> denom -> recip -> rank-1 R -> ets
        slab_h, sct = state[m]
        et = smp.tile([128, 128], F16, tag="et", name="et")
        nc.scalar.activation(
            et[:, :], sct[:, :],
            mybir.ActivationFunctionType.Exp,
            bias=zbias[:, :], scale=SCALE,
        )
        rcx = cx_ps.tile([128, 384], F32, tag="cx", name="rcx")
        r_t = rcx[:, 256:384]
        cxp = rcx[:, 0:256]
        nc.tensor.matmul(r_t[0:1, :], ones_col[:, :], et[:, :])
        reciph = smp.tile([1, 128], F16, tag="reciph", name="reciph")
        with nc.allow_low_precision(reason="softmax recip fits fp16"):
            nc.vector.reciprocal(reciph[:, :], r_t[0:1, :])
        nc.tensor.matmul(r_t[:, :], onesf[0:1, :], reciph[:, :])
        ets = smp.tile([128, 128], F16, tag="ets", name="ets")
        nc.vector.tensor_mul(ets[:, :], et[:, :], r_t[:, :])
        state[m] = (slab_h, ets, cxp)

    def ctx_pairs(m, qls):
        # ctxmT in packed (i, q*8+h) layout: slab chunk stationary (k x 128i),
        # ets columns moving (k x 8h). Emitted interleaved into the NEXT
        # macro's transpose stream so these LDWs hide under its 128-cycle
        # identity streams.
        if _STAGE < 4:
            return
        slab_h, ets, cxp = state[m]
        for ql in qls:
            for ic in range(2):
                nc.tensor.matmul(
                    cxp[:, ic * 128 + ql * 8: ic * 128 + (ql + 1) * 8],
                    slab_h[:, ql * D + ic * 128: ql * D + (ic + 1) * 128],
                    ets[:, ql * 8:(ql + 1) * 8],
                )

    def ctx_flush(m):
        slab_h, ets, cxp = state.pop(m)
        if _STAGE < 5:
            return
        for ic in range(2):
            ceng = nc.vector.tensor_copy if ic == 0 else nc.scalar.copy
            ceng(
                cx_t[ic][:, m * 128:(m + 1) * 128],
                cxp[:, ic * 128:(ic + 1) * 128],
            )

    def front(m, prev):
        b, q0 = m // (LQ // GQ), (m % (LQ // GQ)) * GQ

        # one DMA: (128 k) partitions x one contiguous 8KB run (16q x 256i fp16)
        slab_h = slabp.tile([128, GQ * D], F16, tag="slab", name="slab")
        nc.sync.dma_start(
            out=slab_h.rearrange("k (q i) -> k q i", q=GQ),
            in_=harc_d[b, :, q0:q0 + GQ, :],
        )

        # scoresT (k-part, 16q*8h free); mask first:
        # (lmask.T @ Bsel)[k, q*8+h] = lmask[q, k]
        sct = sc_ps.tile([128, 128], F32, tag="sc_ps", name="sc_ps")
        nc.tensor.matmul(
            sct[:, :], lmask_all[:, m * LK:(m + 1) * LK], bsel[:, :],
            start=True, stop=False,
        )

        for p2 in range(GQ // 2):
            # two queries' transposes (4 chunks) land in one PSUM tile ->
            # one (128,512) copy; fewer ops amortize the per-op overhead
            tpp = tp_ps.tile([128, 512], F16, tag="tp_ps", name="tp_ps")
            for j in range(2):
                ql = p2 * 2 + j
                for ic in range(2):
                    nc.tensor.transpose(
                        tpp[:, j * 256 + ic * 128: j * 256 + (ic + 1) * 128],
                        slab_h[:, ql * D + ic * 128: ql * D + (ic + 1) * 128],
                        ident_h[:, :],
                    )
            tb = tpbp.tile([128, 512], F16, tag="tpb", name="tpb")
            # 5/3 DVE/ACT split of PSUM->SBUF copies (ACT is ~1.5x slower)
            ceng = nc.scalar.copy if p2 in (3, 6) else nc.vector.tensor_copy
            ceng(tb[:, :], tpp[:, :])
            for j in range(2):
                ql = p2 * 2 + j
                nq = m * GQ + ql
                for ic in range(2):
                    nc.tensor.matmul(
                        sct[:, ql * 8:(ql + 1) * 8],
                        tb[:, j * 256 + ic * 128: j * 256 + (ic + 1) * 128],
                        qk_t[ic][:, nq * 8:(nq + 1) * 8],
                        start=False, stop=(ql == GQ - 1 and ic == 1),
                    )
        state[m] = (slab_h, sct)

    # software-pipelined emission, tail depth 2: macro m's softmax chain is
    # emitted before front(m+1) (a full front of runway), and m's context
    # matmuls are interleaved into front(m+2)'s transpose stream so their
    # LDWs hide under the 128-cycle identity streams.
    for m in range(NMACRO):
        if m >= 1 and _STAGE >= 3:
            chain(m - 1)
        if m >= 2 and _STAGE >= 3:
            ctx_pairs(m - 2, range(GQ))
            ctx_flush(m - 2)
        front(m, None)
    if _STAGE >= 3:
        chain(NMACRO - 1)
        for mm in (NMACRO - 2, NMACRO - 1):
            ctx_pairs(mm, range(GQ))
            ctx_flush(mm)


_NC = None


def _get_nc():
    global _NC
    if _NC is None:
        _NC = _build_program()
    return _NC


def _const_inputs():
    id16 = np.eye(128, dtype=np.float16)
    bsel = np.zeros((16, 128), dtype=np.float16)
    for q in range(16):
        bsel[q, q * 8:(q + 1) * 8] = 1
    ones = np.ones((128, 128), dtype=np.float16)
    return {"c_id16": id16, "c_bsel": bsel, "c_ones": ones}


def make_in_maps(full):
    """Host-side preprocessing + per-core sharding of the full inputs."""
    harc_t = np.ascontiguousarray(
        full["h_arc"].astype(np.float16).transpose(0, 2, 1, 3)
    )  # (B, Lk, Lq, D) fp16
    lmaskh = ((full["mask"].astype(np.float32) - 1.0) * (-MASK_RAW)).astype(
        np.float16
    )  # {0, MASK_RAW}
    consts = _const_inputs()
    in_maps = []
    for c in range(NCORES):
        sl = slice(c * BLOC, (c + 1) * BLOC)
        in_maps.append({
            "h": full["h"][sl],
            "h_arcT": harc_t[sl],
            "lmaskh": lmaskh[sl],
            "Wq": full["Wq"], "Wk": full["Wk"], "Wv": full["Wv"], "Wo": full["Wo"],
            "bq": full["bq"], "bv": full["bv"], "bo": full["bo"],
            **consts,
        })
    return in_maps


def kernel(**inputs):
    nc = _get_nc()
    full = {k: np.ascontiguousarray(v) for k, v in inputs.items()}
    in_maps = make_in_maps(full)
    res = run_bass_kernel_spmd(nc, in_maps, list(range(NCORES)))
    out = np.concatenate([res.results[c]["out"] for c in range(NCORES)], axis=0)
    return out.astype(np.float32)
